# revision 6
# baseline (speedup 1.0000x reference)
"""TopoEncoder Trainium2 kernel (8 NeuronCores, data-parallel over batch).

Two-pass design — the reference's single global scalar (max over the whole
batch's distance tensor) is the only cross-core quantity, and the CC-stream
collective path costs ~65us of pure latency on these axon-tunneled cores
(43.6us kernel-entry barrier + 11us stream gap + 9.5us mesh op). Instead:

  pass 1 (per core, 64 samples): x DMA -> mean over T (DVE/GpSimd add-tree +
    PE pair-matrix fold) -> pairwise channel-L2 distance^2, cast fp16 ->
    split each sample's 25x25 across two partitions ([128, 13*25], PE
    identity matmuls) -> Floyd-Warshall min-max closure in fp16 with PE
    row-pivot extraction (selection ops only, so fp16 rounds each d^2 once)
    -> MST mask (M >= d, exact on fp16 values) -> per-half top-24 of the
    masked upper-tri (max8 + match_replace) -> merge halves (PE) -> top-25
    -> kill the first adjacent duplicate and zero slots with a +BIG sentinel
    (fp16 ties can admit a 25th spurious mask edge whose value duplicates a
    real one; measured output rel err ~1e-5 after this) -> sqrt ->
    deaths [64,25] f32, plus the per-half max of d^2 [128,1].
  host: folds gmax = sqrt(max of the 1024 per-half maxima) into the
    structure-element parameters (pure parameter prep, like the baseline's
    csT/pairmat): C_e = 1e-6 + c2_e*R, U_e = (s2_e/R)^2, Ab_e = exp(-(s1 c1)^2)
    with R = gmax - 1e-6 (global min is the d=sqrt(1e-12) diagonal,
    structurally 1e-6).
  pass 2 (per core): normalize-free structure-element layer on deaths with
    the folded params: out[b,e] = Ab_e * sum_p exp(-U_e (death_p - C_e)^2).
    BIG slots underflow exp to exactly 0.

fp16 matters because DVE's 2x mode needs 2-byte dtypes with packed innermost
access; the FW min and the mask all hit it (the col-broadcast max stays at
1x — access-pattern-bound, not ALU-bound — which is why the [128, 13*25]
split layout halves its free size).
"""

from contextlib import ExitStack

import numpy as np

import bass_rust
import concourse.bass as bass
import concourse.tile as tile
from concourse import mybir
from concourse.bass_utils import run_bass_kernel_spmd

N_CORES = 8
B = 64          # samples per core
C, T, V, E = 3, 128, 25, 64
VV = V * V
HR = 13         # rows per half (h=1 row 12 is a zero pad)
HF = HR * V     # free elems per half (325)
ND = 25         # death slots per sample (24 real + sentinel)
DT = mybir.dt.float32
F16 = mybir.dt.float16
GMIN = 1e-6     # sqrt(1e-12): the reference's global min (diagonal), exact
BIG = 60000.0   # fp16-representable sentinel; exp(-U(sqrt(BIG)-C)^2) == 0
VSP = 19        # DVE/GpSimd free-dim split (rate ratio ~1.25 : 4.0 ns/elem)


def _split_excess_waits(nc, cap=1):
    """The walrus build in this env rejects instructions carrying more than
    ~2 semaphore-wait commands. Move excess waits onto same-engine NOPs
    inserted immediately before the offending instruction."""
    n_split = 0
    for bb in nc.main_func.blocks:
        insts = bb.instructions
        i = 0
        while i < len(insts):
            ins = insts[i]
            si = ins.sync_info
            waits = list(si.on_wait) if si and si.on_wait else []
            if len(waits) > cap:
                extra, keep = waits[:-cap], waits[-cap:]
                ins.sync_info = mybir.SyncInfo(
                    on_wait=keep, on_update=list(si.on_update or [])
                )
                for j, w in enumerate(extra):
                    nop = bass_rust.InstNoOp(
                        name=f"I-wsplit-{n_split}-{j}",
                        engine=ins.engine,
                        sync_info=mybir.SyncInfo(on_wait=[w], on_update=[]),
                    )
                    insts.insert(i, nop)
                    i += 1
                n_split += 1
            i += 1
    return n_split


def _build_pass1():
    A = mybir.AluOpType
    ACT = mybir.ActivationFunctionType
    nc = bass.Bass("TRN2", debug=False, num_devices=N_CORES)

    x_in = nc.dram_tensor("x", [B, C, T, V], DT, kind="ExternalInput").ap()
    pm_in = nc.dram_tensor("pm", [128, B], DT, kind="ExternalInput").ap()
    ut_in = nc.dram_tensor("ut2", [128, HF], F16, kind="ExternalInput").ap()
    dup_in = nc.dram_tensor("dup", [128, 128], F16, kind="ExternalInput").ap()
    id_in = nc.dram_tensor("id64", [128, B], F16, kind="ExternalInput").ap()
    dth_d = nc.dram_tensor("deaths", [B, ND], DT, kind="ExternalOutput").ap()
    pmx_d = nc.dram_tensor("pmax", [128, 1], F16, kind="ExternalOutput").ap()

    with tile.TileContext(nc, num_cores=N_CORES) as tc, ExitStack() as ctx:
        sb = ctx.enter_context(tc.tile_pool(name="sb", bufs=1))
        psum = ctx.enter_context(tc.tile_pool(name="psum", bufs=1, space="PSUM"))
        psr = ctx.enter_context(tc.tile_pool(name="psr", bufs=2, space="PSUM"))

        # ---- x DMA first: partition p = t2*64 + b, free = (c, t32, v) ----
        xa = sb.tile([128, C, T // 4, V], DT)
        xb = sb.tile([128, C, T // 4, V], DT)
        nc.sync.dma_start(xa[0:B], x_in[:, :, 0:32, :])
        nc.scalar.dma_start(xa[B:128], x_in[:, :, 64:96, :])
        nc.sync.dma_start(xb[0:B], x_in[:, :, 32:64, :])
        nc.scalar.dma_start(xb[B:128], x_in[:, :, 96:128, :])

        # ---- small constant loads (after x on both queues) ----
        pm_t = sb.tile([128, B], DT)
        nc.sync.dma_start(pm_t[:], pm_in[:])
        ut2 = sb.tile([128, HF], F16)
        nc.scalar.dma_start(ut2[:], ut_in[:])
        dup16 = sb.tile([128, 128], F16)
        nc.sync.dma_start(dup16[:], dup_in[:])
        id16 = sb.tile([128, B], F16)
        nc.scalar.dma_start(id16[:], id_in[:])

        # ---- mean over T: add trees (v-split DVE | GpSimd), PE pair fold ----
        for xh in (xa, xb):
            for w in (16, 8, 4, 2, 1):
                nc.vector.tensor_tensor(
                    out=xh[:, :, 0:w, 0:VSP],
                    in0=xh[:, :, 0:w, 0:VSP],
                    in1=xh[:, :, w : 2 * w, 0:VSP],
                    op=A.add,
                )
                nc.gpsimd.tensor_tensor(
                    out=xh[:, :, 0:w, VSP:V],
                    in0=xh[:, :, 0:w, VSP:V],
                    in1=xh[:, :, w : 2 * w, VSP:V],
                    op=A.add,
                )
        nc.vector.tensor_tensor(
            out=xa[:, :, 0:1, 0:VSP], in0=xa[:, :, 0:1, 0:VSP],
            in1=xb[:, :, 0:1, 0:VSP], op=A.add,
        )
        nc.gpsimd.tensor_tensor(
            out=xa[:, :, 0:1, VSP:V], in0=xa[:, :, 0:1, VSP:V],
            in1=xb[:, :, 0:1, VSP:V], op=A.add,
        )
        ps_xm = psum.tile([B, C, V], DT)
        nc.tensor.matmul(out=ps_xm[:], lhsT=pm_t[:], rhs=xa[:, :, 0, :],
                         start=True, stop=True)
        xm = sb.tile([B, C, V], DT)
        nc.vector.tensor_copy(xm[:], ps_xm[:])

        # ---- distance^2 matrix (no sqrt needed before the deaths) ----
        df = sb.tile([B, C, V, V], DT)
        xmb_i = xm.unsqueeze(-1).broadcast_to([B, C, V, V])
        xmb_j = xm.unsqueeze(2).broadcast_to([B, C, V, V])
        nc.vector.tensor_tensor(
            out=df[:, :, :, 0:VSP], in0=xmb_i[:, :, :, 0:VSP],
            in1=xmb_j[:, :, :, 0:VSP], op=A.subtract,
        )
        nc.gpsimd.tensor_tensor(
            out=df[:, :, :, VSP:V], in0=xmb_i[:, :, :, VSP:V],
            in1=xmb_j[:, :, :, VSP:V], op=A.subtract,
        )
        nc.scalar.square(df[:, 0:2], df[:, 0:2])
        nc.vector.tensor_tensor(out=df[:, 2], in0=df[:, 2], in1=df[:, 2], op=A.mult)
        d12 = sb.tile([B, VV], DT)
        d123 = d12.rearrange("p (i j) -> p i j", i=V)
        nc.vector.tensor_tensor(out=d123[:], in0=df[:, 0], in1=df[:, 1], op=A.add)
        dq64 = sb.tile([B, 2 * HF], F16)
        nc.vector.memset(dq64[:, VV : 2 * HF], 0.0)
        dq643 = dq64[:, 0:VV].rearrange("p (i j) -> p i j", i=V)
        nc.vector.tensor_tensor(out=dq643[:], in0=d123[:], in1=df[:, 2], op=A.add)

        # ---- split to [128, 325]: partitions (h*64+b), rows h*13..h*13+12 ----
        ps_dq = psum.tile([128, HF], DT)
        nc.tensor.matmul(out=ps_dq[0:B, :], lhsT=id16[0:B], rhs=dq64[:, 0:HF],
                         start=True, stop=True)
        nc.tensor.matmul(out=ps_dq[B:128, :], lhsT=id16[0:B], rhs=dq64[:, HF : 2 * HF],
                         start=True, stop=True)
        dq = sb.tile([128, HF], F16)
        nc.vector.tensor_copy(dq[:], ps_dq[:])
        M = sb.tile([128, HF], F16)
        nc.vector.tensor_copy(M[:], dq[:])

        # ---- premasked values (upper-tri pattern, pad row zeroed) ----
        dut = sb.tile([128, HF], F16)
        nc.vector.tensor_tensor(out=dut[:], in0=dq[:], in1=ut2[:], op=A.mult)

        # ---- Floyd-Warshall min-max closure, fp16, PE row-pivot extract ----
        M3 = M.rearrange("p (i j) -> p i j", i=HR)
        fwt = sb.tile([128, HR, V], F16)
        rowk0 = psr.tile([128, V], DT, tag="rowk0")
        rowk1 = psr.tile([128, V], DT, tag="rowk1")
        rowk = [rowk0, rowk1]
        for k in range(V):
            hk, ilk = divmod(k, HR)
            rk = rowk[k % 2]
            nc.tensor.matmul(
                out=rk[:], lhsT=dup16[hk * B : (hk + 1) * B, :],
                rhs=M3[hk * B : (hk + 1) * B, ilk, :],
                start=True, stop=True,
            )
            nc.vector.tensor_tensor(
                out=fwt[:],
                in0=M3[:, :, k : k + 1].broadcast_to([128, HR, V]),
                in1=rk.unsqueeze(1).broadcast_to([128, HR, V]),
                op=A.max,
            )
            if k < V - 1:
                hn, iln = divmod(k + 1, HR)
                # strip for the next pivot row first, so its PE extract can
                # overlap the remaining min update
                nc.vector.tensor_tensor(
                    out=M3[hn * B : (hn + 1) * B, iln : iln + 1, :],
                    in0=M3[hn * B : (hn + 1) * B, iln : iln + 1, :],
                    in1=fwt[hn * B : (hn + 1) * B, iln : iln + 1, :],
                    op=A.min,
                )
                if iln > 0:
                    nc.vector.tensor_tensor(
                        out=M3[:, 0:iln, :], in0=M3[:, 0:iln, :],
                        in1=fwt[:, 0:iln, :], op=A.min,
                    )
                if iln < HR - 1:
                    nc.vector.tensor_tensor(
                        out=M3[:, iln + 1 : HR, :], in0=M3[:, iln + 1 : HR, :],
                        in1=fwt[:, iln + 1 : HR, :], op=A.min,
                    )
                oh = 1 - hn
                nc.vector.tensor_tensor(
                    out=M3[oh * B : (oh + 1) * B, iln : iln + 1, :],
                    in0=M3[oh * B : (oh + 1) * B, iln : iln + 1, :],
                    in1=fwt[oh * B : (oh + 1) * B, iln : iln + 1, :],
                    op=A.min,
                )
            else:
                nc.vector.tensor_tensor(out=M3[:], in0=M3[:], in1=fwt[:], op=A.min)

        # ---- MST mask + masked upper-tri values ----
        mk = sb.tile([128, HF], F16)
        nc.vector.tensor_tensor(out=mk[:], in0=M[:], in1=dq[:], op=A.is_ge)
        val = sb.tile([128, HF], F16)
        nc.vector.tensor_tensor(out=val[:], in0=mk[:], in1=dut[:], op=A.mult)

        # ---- per-half top-24, merge, top-25 ----
        d24h = sb.tile([128, 24], F16)
        mr1 = sb.tile([128, HF], F16)
        mr2 = sb.tile([128, HF], F16)
        nc.vector.max(d24h[:, 0:8], val[:])
        nc.vector.match_replace(mr1[:], d24h[:, 0:8], val[:], 0.0)
        nc.vector.max(d24h[:, 8:16], mr1[:])
        nc.vector.match_replace(mr2[:], d24h[:, 8:16], mr1[:], 0.0)
        nc.vector.max(d24h[:, 16:24], mr2[:])

        ps_c = psum.tile([B, 48], DT)
        nc.tensor.matmul(out=ps_c[:, 0:24], lhsT=id16[0:B], rhs=d24h[0:B, :],
                         start=True, stop=True)
        nc.tensor.matmul(out=ps_c[:, 24:48], lhsT=id16[B:128], rhs=d24h[B:128, :],
                         start=True, stop=True)
        cand = sb.tile([B, 48], F16)
        nc.vector.tensor_copy(cand[:], ps_c[:])

        d25 = sb.tile([B, 32], F16)
        cr1 = sb.tile([B, 48], F16)
        cr2 = sb.tile([B, 48], F16)
        cr3 = sb.tile([B, 48], F16)
        nc.vector.max(d25[:, 0:8], cand[:])
        nc.vector.match_replace(cr1[:], d25[:, 0:8], cand[:], 0.0)
        nc.vector.max(d25[:, 8:16], cr1[:])
        nc.vector.match_replace(cr2[:], d25[:, 8:16], cr1[:], 0.0)
        nc.vector.max(d25[:, 16:24], cr2[:])
        nc.vector.match_replace(cr3[:], d25[:, 16:24], cr2[:], 0.0)
        nc.vector.max(d25[:, 24:32], cr3[:])

        # ---- fp16-tie dedup: kill the first adjacent duplicate (only when a
        # 25th nonzero exists), then map empty slots to BIG ----
        bigt = sb.tile([B, 1], DT)
        nc.vector.memset(bigt[:], BIG)
        g32 = sb.tile([B, 1], DT)
        zt = sb.tile([B, 1], F16)
        nc.vector.memset(zt[:], 0.0)
        nc.vector.tensor_tensor(out=g32[:], in0=d25[:, 24:25], in1=zt[:], op=A.is_gt)
        eq = sb.tile([B, 24], DT)
        nc.vector.tensor_tensor(out=eq[:], in0=d25[:, 0:24], in1=d25[:, 1:25],
                                op=A.is_equal)
        nc.vector.tensor_scalar(out=eq[:], in0=eq[:], scalar1=g32[:, 0:1],
                                scalar2=None, op0=A.mult)
        # prefix-sum of eq via shift-adds (ping-pong)
        csa = sb.tile([B, 24], DT)
        csb = sb.tile([B, 24], DT)
        nc.vector.tensor_copy(csa[:], eq[:])
        src, dst = csa, csb
        for s in (1, 2, 4, 8, 16):
            nc.vector.tensor_copy(dst[:], src[:])
            nc.vector.tensor_tensor(out=dst[:, s:24], in0=dst[:, s:24],
                                    in1=src[:, 0 : 24 - s], op=A.add)
            src, dst = dst, src
        one_t = sb.tile([B, 1], DT)
        nc.vector.memset(one_t[:], 1.0)
        km = sb.tile([B, 24], DT)
        nc.vector.tensor_scalar(out=km[:], in0=src[:], scalar1=one_t[:, 0:1],
                                scalar2=None, op0=A.is_equal)
        nc.vector.tensor_tensor(out=km[:], in0=km[:], in1=eq[:], op=A.mult)
        nc.vector.scalar_tensor_tensor(
            out=d25[:, 0:24], in0=km[:], scalar=bigt[:, 0:1], in1=d25[:, 0:24],
            op0=A.mult, op1=A.add,
        )
        zb = sb.tile([B, ND], DT)
        nc.vector.tensor_scalar(out=zb[:], in0=d25[:, 0:ND], scalar1=0.0,
                                scalar2=None, op0=A.is_equal)
        nc.vector.scalar_tensor_tensor(
            out=d25[:, 0:ND], in0=zb[:], scalar=bigt[:, 0:1], in1=d25[:, 0:ND],
            op0=A.mult, op1=A.add,
        )

        # ---- deaths = sqrt(selected d^2), fp32 out ----
        dth = sb.tile([B, ND], DT)
        nc.scalar.activation(dth[:], d25[:, 0:ND], ACT.Sqrt, bias=0.0, scale=1.0)
        nc.sync.dma_start(dth_d[:], dth[:])

        # ---- per-half max of d^2 (host folds into the global max) ----
        pmx = sb.tile([128, 1], F16)
        nc.vector.tensor_reduce(out=pmx[:], in_=dq[:],
                                axis=mybir.AxisListType.X, op=A.max)
        nc.scalar.dma_start(pmx_d[:], pmx[:])

    _split_excess_waits(nc)
    return nc


def _build_pass2():
    A = mybir.AluOpType
    ACT = mybir.ActivationFunctionType
    nc = bass.Bass("TRN2", debug=False, num_devices=N_CORES)

    dth_in = nc.dram_tensor("deaths", [B, ND], DT, kind="ExternalInput").ap()
    prm_in = nc.dram_tensor("prm", [1, 3 * E], DT, kind="ExternalInput").ap()
    out_d = nc.dram_tensor("out", [B, E], DT, kind="ExternalOutput").ap()

    with tile.TileContext(nc, num_cores=N_CORES) as tc, ExitStack() as ctx:
        sb = ctx.enter_context(tc.tile_pool(name="sb", bufs=1))
        work = ctx.enter_context(tc.tile_pool(name="work", bufs=2))
        psum = ctx.enter_context(tc.tile_pool(name="psum", bufs=1, space="PSUM"))

        dth = sb.tile([B, ND], DT)
        nc.sync.dma_start(dth[:], dth_in[:])
        prow = sb.tile([1, 3 * E], DT)
        nc.scalar.dma_start(prow[:], prm_in[:])
        ones1 = sb.tile([1, B], DT)
        nc.vector.memset(ones1[:], 1.0)

        # broadcast params to all partitions: [B, 3, E] = (C_e, U_e, Ab_e)
        prm = psum.tile([B, 3, E], DT)
        nc.tensor.matmul(out=prm[:], lhsT=ones1[:], rhs=prow[:], start=True, stop=True)
        Cb = sb.tile([B, E], DT)
        nc.vector.tensor_copy(Cb[:], prm[:, 0, :])
        Ub = sb.tile([B, E], DT)
        nc.vector.tensor_copy(Ub[:], prm[:, 1, :])
        Ab = sb.tile([B, E], DT)
        nc.vector.tensor_copy(Ab[:], prm[:, 2, :])

        # structure element layer: out[b,e] = Ab_e * sum_p exp(-U_e (dth - C_e)^2)
        S = sb.tile([B, E], DT)
        ECH = 16
        for ch in range(E // ECH):
            e0 = ch * ECH
            t1 = work.tile([B, ECH, ND], DT, tag="t1")
            nc.vector.tensor_tensor(
                out=t1[:],
                in0=dth.unsqueeze(1).broadcast_to([B, ECH, ND]),
                in1=Cb[:, e0 : e0 + ECH].unsqueeze(-1).broadcast_to([B, ECH, ND]),
                op=A.subtract,
            )
            nc.scalar.square(t1[:], t1[:])
            nc.vector.tensor_tensor(
                out=t1[:],
                in0=t1[:],
                in1=Ub[:, e0 : e0 + ECH].unsqueeze(-1).broadcast_to([B, ECH, ND]),
                op=A.mult,
            )
            fexp = work.tile([B, ECH, ND], DT, tag="fexp")
            nc.scalar.activation(fexp[:], t1[:], ACT.Exp, bias=0.0, scale=-1.0)
            nc.vector.tensor_reduce(
                out=S[:, e0 : e0 + ECH], in_=fexp[:], axis=mybir.AxisListType.X,
                op=A.add,
            )
        outt = sb.tile([B, E], DT)
        nc.vector.tensor_tensor(out=outt[:], in0=S[:], in1=Ab[:], op=A.mult)
        nc.sync.dma_start(out_d[:], outt[:])

    _split_excess_waits(nc)
    return nc


_CACHE = {}


def _consts():
    # pair matrix: adds partition rows b and b+64 (the two T-halves) and
    # applies the 1/T mean scale
    pairmat = np.zeros((128, B), dtype=np.float32)
    for p in range(128):
        pairmat[p, p % B] = 1.0 / T
    # upper-tri premask in the split layout: partition (h*64+b) holds rows
    # i = h*13 .. h*13+12; pad row (h=1, il=12) is zero
    ut2 = np.zeros((128, HF), dtype=np.float16)
    for h in range(2):
        for il in range(HR):
            i = h * HR + il
            if i >= V:
                continue
            for j in range(V):
                if j > i:
                    ut2[h * B : (h + 1) * B, il * V + j] = 1.0
    dup = np.zeros((128, 128), dtype=np.float16)
    for p in range(128):
        dup[p % B, p] = 1.0
        dup[B + p % B, p] = 1.0
    id64 = np.concatenate([np.eye(B, dtype=np.float16)] * 2, axis=0)
    return pairmat, ut2, dup, id64


def _get_programs():
    if "p1" not in _CACHE:
        _CACHE["p1"] = _build_pass1()
        _CACHE["p2"] = _build_pass2()
    return _CACHE["p1"], _CACHE["p2"]


def _run(x, centres, sharpness, **run_kwargs):
    p1, p2 = _get_programs()
    xf = np.ascontiguousarray(x.reshape(-1, C, T, V)).astype(np.float32, copy=False)
    n_total = xf.shape[0]
    assert n_total == N_CORES * B, xf.shape
    pairmat, ut2, dup, id64 = _consts()

    in1 = [
        {
            "x": np.ascontiguousarray(xf[i * B : (i + 1) * B]),
            "pm": pairmat,
            "ut2": ut2,
            "dup": dup,
            "id64": id64,
        }
        for i in range(N_CORES)
    ]
    res1 = run_bass_kernel_spmd(p1, in1, list(range(N_CORES)), **run_kwargs)

    # host: fold the global max into the structure-element parameters
    gmax2 = max(
        float(np.max(res1.results[i]["pmax"].astype(np.float32)))
        for i in range(N_CORES)
    )
    gmax = float(np.sqrt(gmax2))
    R = gmax - GMIN
    c1 = centres[:, 0].astype(np.float64)
    c2 = centres[:, 1].astype(np.float64)
    s1 = sharpness[:, 0].astype(np.float64)
    s2 = sharpness[:, 1].astype(np.float64)
    Ce = GMIN + c2 * R
    Ue = (s2 / R) ** 2
    Abe = np.exp(-((s1 * c1) ** 2))
    prm = np.ascontiguousarray(
        np.stack([Ce, Ue, Abe], axis=0).astype(np.float32).reshape(1, 3 * E)
    )

    in2 = [
        {"deaths": np.ascontiguousarray(res1.results[i]["deaths"]), "prm": prm}
        for i in range(N_CORES)
    ]
    res2 = run_bass_kernel_spmd(p2, in2, list(range(N_CORES)), **run_kwargs)

    out = np.concatenate([res2.results[i]["out"] for i in range(N_CORES)], axis=0)
    return out, (res1, res2)


def kernel(x, centres, sharpness):
    out, _ = _run(np.asarray(x), np.asarray(centres), np.asarray(sharpness))
    return out


# revision 8
# speedup vs baseline: 1.0926x; 1.0926x over previous
"""TopoEncoder Trainium2 kernel (8 NeuronCores, data-parallel over batch).

Two-pass design — the reference's single global scalar (max over the whole
batch's distance tensor) is the only cross-core quantity, and the CC-stream
collective path costs ~65us of pure latency on these axon-tunneled cores
(43.6us kernel-entry barrier + 11us stream gap + 9.5us mesh op). Instead:

  pass 1 (per core, 64 samples): x DMA -> mean over T (DVE/GpSimd add-tree +
    PE pair-matrix fold) -> pairwise channel-L2 distance^2, cast fp16 ->
    split each sample's 25x25 across two partitions ([128, 13*25], PE
    identity matmuls) -> Floyd-Warshall min-max closure in fp16 with PE
    row-pivot extraction (selection ops only, so fp16 rounds each d^2 once)
    -> MST mask (M >= d, exact on fp16 values) -> per-half top-24 of the
    masked upper-tri (max8 + match_replace) -> merge halves (PE) -> top-25
    -> kill the first adjacent duplicate and zero slots with a +BIG sentinel
    (fp16 ties can admit a 25th spurious mask edge whose value duplicates a
    real one; measured output rel err ~1e-5 after this) -> sqrt ->
    deaths [64,25] f32, plus the per-half max of d^2 [128,1].
  host: folds gmax = sqrt(max of the 1024 per-half maxima) into the
    structure-element parameters (pure parameter prep, like the baseline's
    csT/pairmat): C_e = 1e-6 + c2_e*R, U_e = (s2_e/R)^2, Ab_e = exp(-(s1 c1)^2)
    with R = gmax - 1e-6 (global min is the d=sqrt(1e-12) diagonal,
    structurally 1e-6).
  pass 2 (per core): normalize-free structure-element layer on deaths with
    the folded params: out[b,e] = Ab_e * sum_p exp(-U_e (death_p - C_e)^2).
    BIG slots underflow exp to exactly 0.

fp16 matters because DVE's 2x mode needs 2-byte dtypes with packed innermost
access; the FW min and the mask all hit it (the col-broadcast max stays at
1x — access-pattern-bound, not ALU-bound — which is why the [128, 13*25]
split layout halves its free size).
"""

from contextlib import ExitStack

import numpy as np

import bass_rust
import concourse.bass as bass
import concourse.tile as tile
from concourse import mybir
from concourse.bass_utils import run_bass_kernel_spmd

N_CORES = 8
B = 64          # samples per core
C, T, V, E = 3, 128, 25, 64
VV = V * V
HR = 13         # rows per half (h=1 row 12 is a zero pad)
HF = HR * V     # free elems per half (325)
ND = 25         # death slots per sample (24 real + sentinel)
DT = mybir.dt.float32
F16 = mybir.dt.float16
GMIN = 1e-6     # sqrt(1e-12): the reference's global min (diagonal), exact
BIG = 60000.0   # fp16-representable sentinel; exp(-U(sqrt(BIG)-C)^2) == 0
VSP = 19        # DVE/GpSimd free-dim split (rate ratio ~1.25 : 4.0 ns/elem)


def _split_excess_waits(nc, cap=1):
    """The walrus build in this env rejects instructions carrying more than
    ~2 semaphore-wait commands. Move excess waits onto same-engine NOPs
    inserted immediately before the offending instruction."""
    n_split = 0
    for bb in nc.main_func.blocks:
        insts = bb.instructions
        i = 0
        while i < len(insts):
            ins = insts[i]
            si = ins.sync_info
            waits = list(si.on_wait) if si and si.on_wait else []
            if len(waits) > cap:
                extra, keep = waits[:-cap], waits[-cap:]
                ins.sync_info = mybir.SyncInfo(
                    on_wait=keep, on_update=list(si.on_update or [])
                )
                for j, w in enumerate(extra):
                    nop = bass_rust.InstNoOp(
                        name=f"I-wsplit-{n_split}-{j}",
                        engine=ins.engine,
                        sync_info=mybir.SyncInfo(on_wait=[w], on_update=[]),
                    )
                    insts.insert(i, nop)
                    i += 1
                n_split += 1
            i += 1
    return n_split


def _build_pass1():
    A = mybir.AluOpType
    ACT = mybir.ActivationFunctionType
    nc = bass.Bass("TRN2", debug=False, num_devices=N_CORES)

    x_in = nc.dram_tensor("x", [B, C, T, V], DT, kind="ExternalInput").ap()
    pm_in = nc.dram_tensor("pm", [128, B], DT, kind="ExternalInput").ap()
    ut_in = nc.dram_tensor("ut2", [128, HF], F16, kind="ExternalInput").ap()
    dup_in = nc.dram_tensor("dup", [128, 128], F16, kind="ExternalInput").ap()
    id_in = nc.dram_tensor("id64", [128, B], F16, kind="ExternalInput").ap()
    dth_d = nc.dram_tensor("deaths", [B, ND], DT, kind="ExternalOutput").ap()
    pmx_d = nc.dram_tensor("pmax", [128, 1], F16, kind="ExternalOutput").ap()

    with tile.TileContext(nc, num_cores=N_CORES) as tc, ExitStack() as ctx:
        sb = ctx.enter_context(tc.tile_pool(name="sb", bufs=1))
        psum = ctx.enter_context(tc.tile_pool(name="psum", bufs=1, space="PSUM"))
        psr = ctx.enter_context(tc.tile_pool(name="psr", bufs=2, space="PSUM"))

        # ---- x DMA on 4 queues: xa first (tree starts on it), then xb ----
        xa = sb.tile([128, C, T // 4, V], DT)
        xb = sb.tile([128, C, T // 4, V], DT)
        nc.sync.dma_start(xa[0:B], x_in[:, :, 0:32, :])
        nc.scalar.dma_start(xa[B:128], x_in[:, :, 64:96, :])
        nc.sync.dma_start(xb[0:B], x_in[:, :, 32:64, :])
        nc.scalar.dma_start(xb[B:128], x_in[:, :, 96:128, :])

        # ---- small constant loads (after x) ----
        pm_t = sb.tile([128, B], DT)
        nc.sync.dma_start(pm_t[:], pm_in[:])
        ut2 = sb.tile([128, HF], F16)
        nc.scalar.dma_start(ut2[:], ut_in[:])
        dup16 = sb.tile([128, 128], F16)
        nc.sync.dma_start(dup16[:], dup_in[:])
        id16 = sb.tile([128, B], F16)
        nc.scalar.dma_start(id16[:], id_in[:])

        # ---- preload activation tables during the DMA gap ----
        warm = sb.tile([1, 2], DT)
        nc.vector.memset(warm[:], 1.0)
        nc.scalar.square(warm[:], warm[:])
        nc.scalar.activation(warm[:], warm[:], ACT.Sqrt, bias=0.0, scale=1.0)

        # ---- mean over T: all-DVE add tree; level 16 split per DMA chunk ----
        for xh in (xa, xb):
            for pb in (0, B):
                nc.vector.tensor_tensor(
                    out=xh[pb : pb + B, :, 0:16, :],
                    in0=xh[pb : pb + B, :, 0:16, :],
                    in1=xh[pb : pb + B, :, 16:32, :],
                    op=A.add,
                )
            for w in (8, 4, 2, 1):
                nc.vector.tensor_tensor(
                    out=xh[:, :, 0:w, :],
                    in0=xh[:, :, 0:w, :],
                    in1=xh[:, :, w : 2 * w, :],
                    op=A.add,
                )
        nc.vector.tensor_tensor(
            out=xa[:, :, 0:1, :], in0=xa[:, :, 0:1, :], in1=xb[:, :, 0:1, :],
            op=A.add,
        )
        ps_xm = psum.tile([B, C, V], DT)
        nc.tensor.matmul(out=ps_xm[:], lhsT=pm_t[:], rhs=xa[:, :, 0, :],
                         start=True, stop=True)
        xm = sb.tile([B, C, V], DT)
        nc.vector.tensor_copy(xm[:], ps_xm[:])

        # ---- distance^2 matrix (no sqrt needed before the deaths) ----
        df = sb.tile([B, C, V, V], DT)
        xmb_i = xm.unsqueeze(-1).broadcast_to([B, C, V, V])
        xmb_j = xm.unsqueeze(2).broadcast_to([B, C, V, V])
        nc.vector.tensor_tensor(out=df[:], in0=xmb_i[:], in1=xmb_j[:], op=A.subtract)
        nc.scalar.square(df[:, 0:2], df[:, 0:2])
        nc.vector.tensor_tensor(out=df[:, 2], in0=df[:, 2], in1=df[:, 2], op=A.mult)
        d12 = sb.tile([B, VV], DT)
        d123 = d12.rearrange("p (i j) -> p i j", i=V)
        nc.vector.tensor_tensor(out=d123[:], in0=df[:, 0], in1=df[:, 1], op=A.add)
        dq64 = sb.tile([B, 2 * HF], F16)
        nc.vector.memset(dq64[:, VV : 2 * HF], 0.0)
        dq643 = dq64[:, 0:VV].rearrange("p (i j) -> p i j", i=V)
        nc.vector.tensor_tensor(out=dq643[:], in0=d123[:], in1=df[:, 2], op=A.add)

        # ---- split to [128, 325]: partitions (h*64+b), rows h*13..h*13+12 ----
        ps_dq = psum.tile([128, HF], DT)
        nc.tensor.matmul(out=ps_dq[0:B, :], lhsT=id16[0:B], rhs=dq64[:, 0:HF],
                         start=True, stop=True)
        nc.tensor.matmul(out=ps_dq[B:128, :], lhsT=id16[0:B], rhs=dq64[:, HF : 2 * HF],
                         start=True, stop=True)
        dq = sb.tile([128, HF], F16)
        nc.vector.tensor_copy(dq[:], ps_dq[:])
        M = sb.tile([128, HF], F16)
        nc.vector.tensor_copy(M[:], dq[:])

        # ---- Floyd-Warshall min-max closure, fp16, PE row-pivot extract.
        # Per step: one 1x broadcast max, one tiny strip-copy min (feeds the
        # next pivot's PE broadcast without a WAR on M), one full 2x min.
        M3 = M.rearrange("p (i j) -> p i j", i=HR)
        fwt = sb.tile([128, HR, V], F16)
        stripc = sb.tile([128, V], F16)
        rowk0 = psr.tile([128, V], DT, tag="rowk0")
        rowk1 = psr.tile([128, V], DT, tag="rowk1")
        rowk = [rowk0, rowk1]
        for k in range(V):
            hk, ilk = divmod(k, HR)
            rk = rowk[k % 2]
            if k == 0:
                rhs = M3[0:B, 0, :]
            else:
                rhs = stripc[hk * B : (hk + 1) * B, :]
            nc.tensor.matmul(out=rk[:], lhsT=dup16[hk * B : (hk + 1) * B, :],
                             rhs=rhs, start=True, stop=True)
            nc.vector.tensor_tensor(
                out=fwt[:],
                in0=M3[:, :, k : k + 1].broadcast_to([128, HR, V]),
                in1=rk.unsqueeze(1).broadcast_to([128, HR, V]),
                op=A.max,
            )
            if k < V - 1:
                hn, iln = divmod(k + 1, HR)
                nc.vector.tensor_tensor(
                    out=stripc[hn * B : (hn + 1) * B, :],
                    in0=M3[hn * B : (hn + 1) * B, iln, :],
                    in1=fwt[hn * B : (hn + 1) * B, iln, :],
                    op=A.min,
                )
            nc.vector.tensor_tensor(out=M3[:], in0=M3[:], in1=fwt[:], op=A.min)

        # ---- premask, MST mask, masked values ----
        dut = sb.tile([128, HF], F16)
        nc.vector.tensor_tensor(out=dut[:], in0=dq[:], in1=ut2[:], op=A.mult)
        mk = sb.tile([128, HF], F16)
        nc.vector.tensor_tensor(out=mk[:], in0=M[:], in1=dq[:], op=A.is_ge)
        val = sb.tile([128, HF], F16)
        nc.vector.tensor_tensor(out=val[:], in0=mk[:], in1=dut[:], op=A.mult)

        # ---- per-half top-24, merge, top-25 ----
        d24h = sb.tile([128, 24], F16)
        mr1 = sb.tile([128, HF], F16)
        mr2 = sb.tile([128, HF], F16)
        nc.vector.max(d24h[:, 0:8], val[:])
        nc.vector.match_replace(mr1[:], d24h[:, 0:8], val[:], 0.0)
        nc.vector.max(d24h[:, 8:16], mr1[:])
        nc.vector.match_replace(mr2[:], d24h[:, 8:16], mr1[:], 0.0)
        nc.vector.max(d24h[:, 16:24], mr2[:])

        ps_c = psum.tile([B, 48], DT)
        nc.tensor.matmul(out=ps_c[:, 0:24], lhsT=id16[0:B], rhs=d24h[0:B, :],
                         start=True, stop=True)
        nc.tensor.matmul(out=ps_c[:, 24:48], lhsT=id16[B:128], rhs=d24h[B:128, :],
                         start=True, stop=True)
        cand = sb.tile([B, 48], F16)
        nc.vector.tensor_copy(cand[:], ps_c[:])

        d25 = sb.tile([B, 32], F16)
        cr1 = sb.tile([B, 48], F16)
        cr2 = sb.tile([B, 48], F16)
        cr3 = sb.tile([B, 48], F16)
        nc.vector.max(d25[:, 0:8], cand[:])
        nc.vector.match_replace(cr1[:], d25[:, 0:8], cand[:], 0.0)
        nc.vector.max(d25[:, 8:16], cr1[:])
        nc.vector.match_replace(cr2[:], d25[:, 8:16], cr1[:], 0.0)
        nc.vector.max(d25[:, 16:24], cr2[:])
        nc.vector.match_replace(cr3[:], d25[:, 16:24], cr2[:], 0.0)
        nc.vector.max(d25[:, 24:32], cr3[:])

        # ---- fp16-tie dedup: when a 25th nonzero exists, kill adjacent
        # duplicates with a +BIG sentinel; then map empty slots to BIG ----
        bigt = sb.tile([B, 1], DT)
        nc.vector.memset(bigt[:], BIG)
        zt = sb.tile([B, 1], F16)
        nc.vector.memset(zt[:], 0.0)
        g32 = sb.tile([B, 1], DT)
        nc.vector.tensor_tensor(out=g32[:], in0=d25[:, 24:25], in1=zt[:], op=A.is_gt)
        eq = sb.tile([B, 24], DT)
        nc.vector.tensor_tensor(out=eq[:], in0=d25[:, 0:24], in1=d25[:, 1:25],
                                op=A.is_equal)
        nc.vector.tensor_scalar(out=eq[:], in0=eq[:], scalar1=g32[:, 0:1],
                                scalar2=None, op0=A.mult)
        nc.vector.scalar_tensor_tensor(
            out=d25[:, 0:24], in0=eq[:], scalar=bigt[:, 0:1], in1=d25[:, 0:24],
            op0=A.mult, op1=A.add,
        )
        zb = sb.tile([B, ND], DT)
        nc.vector.tensor_scalar(out=zb[:], in0=d25[:, 0:ND], scalar1=0.0,
                                scalar2=None, op0=A.is_equal)
        nc.vector.scalar_tensor_tensor(
            out=d25[:, 0:ND], in0=zb[:], scalar=bigt[:, 0:1], in1=d25[:, 0:ND],
            op0=A.mult, op1=A.add,
        )

        # ---- deaths = sqrt(selected d^2), fp32 out ----
        dth = sb.tile([B, ND], DT)
        nc.scalar.activation(dth[:], d25[:, 0:ND], ACT.Sqrt, bias=0.0, scale=1.0)
        nc.sync.dma_start(dth_d[:], dth[:])

        # ---- per-half max of d^2 (host folds into the global max) ----
        pmx = sb.tile([128, 1], F16)
        nc.vector.tensor_reduce(out=pmx[:], in_=dq[:],
                                axis=mybir.AxisListType.X, op=A.max)
        nc.scalar.dma_start(pmx_d[:], pmx[:])

    _split_excess_waits(nc)
    return nc


def _build_pass2():
    A = mybir.AluOpType
    ACT = mybir.ActivationFunctionType
    nc = bass.Bass("TRN2", debug=False, num_devices=N_CORES)

    dth_in = nc.dram_tensor("deaths", [B, ND], DT, kind="ExternalInput").ap()
    prm_in = nc.dram_tensor("prm", [1, 3 * E], DT, kind="ExternalInput").ap()
    out_d = nc.dram_tensor("out", [B, E], DT, kind="ExternalOutput").ap()

    with tile.TileContext(nc, num_cores=N_CORES) as tc, ExitStack() as ctx:
        sb = ctx.enter_context(tc.tile_pool(name="sb", bufs=1))
        work = ctx.enter_context(tc.tile_pool(name="work", bufs=2))
        psum = ctx.enter_context(tc.tile_pool(name="psum", bufs=1, space="PSUM"))

        dth = sb.tile([B, ND], DT)
        nc.sync.dma_start(dth[:], dth_in[:])
        prow = sb.tile([1, 3 * E], DT)
        nc.scalar.dma_start(prow[:], prm_in[:])
        ones1 = sb.tile([1, B], DT)
        nc.vector.memset(ones1[:], 1.0)
        warm = sb.tile([1, 2], DT)
        nc.vector.memset(warm[:], 1.0)
        nc.scalar.activation(warm[:], warm[:], ACT.Exp, bias=0.0, scale=-1.0)
        nc.scalar.square(warm[:], warm[:])

        # broadcast params to all partitions: [B, 3, E] = (C_e, U_e, Ab_e)
        prm = psum.tile([B, 3, E], DT)
        nc.tensor.matmul(out=prm[:], lhsT=ones1[:], rhs=prow[:], start=True, stop=True)
        Cb = sb.tile([B, E], DT)
        nc.vector.tensor_copy(Cb[:], prm[:, 0, :])
        Ub = sb.tile([B, E], DT)
        nc.vector.tensor_copy(Ub[:], prm[:, 1, :])
        Ab = sb.tile([B, E], DT)
        nc.vector.tensor_copy(Ab[:], prm[:, 2, :])

        # structure element layer: out[b,e] = Ab_e * sum_p exp(-U_e (dth - C_e)^2)
        S = sb.tile([B, E], DT)
        ECH = 32
        for ch in range(E // ECH):
            e0 = ch * ECH
            t1 = work.tile([B, ECH, ND], DT, tag="t1")
            nc.vector.tensor_tensor(
                out=t1[:],
                in0=dth.unsqueeze(1).broadcast_to([B, ECH, ND]),
                in1=Cb[:, e0 : e0 + ECH].unsqueeze(-1).broadcast_to([B, ECH, ND]),
                op=A.subtract,
            )
            nc.scalar.square(t1[:], t1[:])
            nc.vector.tensor_tensor(
                out=t1[:],
                in0=t1[:],
                in1=Ub[:, e0 : e0 + ECH].unsqueeze(-1).broadcast_to([B, ECH, ND]),
                op=A.mult,
            )
            fexp = work.tile([B, ECH, ND], DT, tag="fexp")
            nc.scalar.activation(fexp[:], t1[:], ACT.Exp, bias=0.0, scale=-1.0)
            nc.vector.tensor_reduce(
                out=S[:, e0 : e0 + ECH], in_=fexp[:], axis=mybir.AxisListType.X,
                op=A.add,
            )
        outt = sb.tile([B, E], DT)
        nc.vector.tensor_tensor(out=outt[:], in0=S[:], in1=Ab[:], op=A.mult)
        nc.sync.dma_start(out_d[:], outt[:])

    _split_excess_waits(nc)
    return nc


_CACHE = {}


def _consts():
    # pair matrix: adds partition rows b and b+64 (the two T-halves) and
    # applies the 1/T mean scale
    pairmat = np.zeros((128, B), dtype=np.float32)
    for p in range(128):
        pairmat[p, p % B] = 1.0 / T
    # upper-tri premask in the split layout: partition (h*64+b) holds rows
    # i = h*13 .. h*13+12; pad row (h=1, il=12) is zero
    ut2 = np.zeros((128, HF), dtype=np.float16)
    for h in range(2):
        for il in range(HR):
            i = h * HR + il
            if i >= V:
                continue
            for j in range(V):
                if j > i:
                    ut2[h * B : (h + 1) * B, il * V + j] = 1.0
    dup = np.zeros((128, 128), dtype=np.float16)
    for p in range(128):
        dup[p % B, p] = 1.0
        dup[B + p % B, p] = 1.0
    id64 = np.concatenate([np.eye(B, dtype=np.float16)] * 2, axis=0)
    return pairmat, ut2, dup, id64


def _get_programs():
    if "p1" not in _CACHE:
        _CACHE["p1"] = _build_pass1()
        _CACHE["p2"] = _build_pass2()
    return _CACHE["p1"], _CACHE["p2"]


def _run(x, centres, sharpness, **run_kwargs):
    p1, p2 = _get_programs()
    xf = np.ascontiguousarray(x.reshape(-1, C, T, V)).astype(np.float32, copy=False)
    n_total = xf.shape[0]
    assert n_total == N_CORES * B, xf.shape
    pairmat, ut2, dup, id64 = _consts()

    in1 = [
        {
            "x": np.ascontiguousarray(xf[i * B : (i + 1) * B]),
            "pm": pairmat,
            "ut2": ut2,
            "dup": dup,
            "id64": id64,
        }
        for i in range(N_CORES)
    ]
    res1 = run_bass_kernel_spmd(p1, in1, list(range(N_CORES)), **run_kwargs)

    # host: fold the global max into the structure-element parameters
    gmax2 = max(
        float(np.max(res1.results[i]["pmax"].astype(np.float32)))
        for i in range(N_CORES)
    )
    gmax = float(np.sqrt(gmax2))
    R = gmax - GMIN
    c1 = centres[:, 0].astype(np.float64)
    c2 = centres[:, 1].astype(np.float64)
    s1 = sharpness[:, 0].astype(np.float64)
    s2 = sharpness[:, 1].astype(np.float64)
    Ce = GMIN + c2 * R
    Ue = (s2 / R) ** 2
    Abe = np.exp(-((s1 * c1) ** 2))
    prm = np.ascontiguousarray(
        np.stack([Ce, Ue, Abe], axis=0).astype(np.float32).reshape(1, 3 * E)
    )

    in2 = [
        {"deaths": np.ascontiguousarray(res1.results[i]["deaths"]), "prm": prm}
        for i in range(N_CORES)
    ]
    res2 = run_bass_kernel_spmd(p2, in2, list(range(N_CORES)), **run_kwargs)

    out = np.concatenate([res2.results[i]["out"] for i in range(N_CORES)], axis=0)
    return out, (res1, res2)


def kernel(x, centres, sharpness):
    out, _ = _run(np.asarray(x), np.asarray(centres), np.asarray(sharpness))
    return out


# revision 10
# speedup vs baseline: 1.2078x; 1.1055x over previous
"""TopoEncoder Trainium2 kernel (8 NeuronCores, data-parallel over batch).

Two-pass design — the reference's single global scalar (max over the whole
batch's distance tensor) is the only cross-core quantity, and the CC-stream
collective path costs ~65us of pure latency on these axon-tunneled cores
(43.6us kernel-entry barrier + 11us stream gap + 9.5us mesh op). Instead:

  pass 1 (per core, 64 samples): x DMA -> mean over T (DVE/GpSimd add-tree +
    PE pair-matrix fold) -> pairwise channel-L2 distance^2, cast fp16 ->
    split each sample's 25x25 across two partitions ([128, 13*25], PE
    identity matmuls) -> Floyd-Warshall min-max closure in fp16 with PE
    row-pivot extraction (selection ops only, so fp16 rounds each d^2 once)
    -> MST mask (M >= d, exact on fp16 values) -> per-half top-24 of the
    masked upper-tri (max8 + match_replace) -> merge halves (PE) -> top-25
    -> kill the first adjacent duplicate and zero slots with a +BIG sentinel
    (fp16 ties can admit a 25th spurious mask edge whose value duplicates a
    real one; measured output rel err ~1e-5 after this) -> sqrt ->
    deaths [64,25] f32, plus the per-half max of d^2 [128,1].
  host: folds gmax = sqrt(max of the 1024 per-half maxima) into the
    structure-element parameters (pure parameter prep, like the baseline's
    csT/pairmat): C_e = 1e-6 + c2_e*R, U_e = (s2_e/R)^2, Ab_e = exp(-(s1 c1)^2)
    with R = gmax - 1e-6 (global min is the d=sqrt(1e-12) diagonal,
    structurally 1e-6).
  pass 2 (per core): normalize-free structure-element layer on deaths with
    the folded params: out[b,e] = Ab_e * sum_p exp(-U_e (death_p - C_e)^2).
    BIG slots underflow exp to exactly 0.

fp16 matters because DVE's 2x mode needs 2-byte dtypes with packed innermost
access; the FW min and the mask all hit it (the col-broadcast max stays at
1x — access-pattern-bound, not ALU-bound — which is why the [128, 13*25]
split layout halves its free size).
"""

from contextlib import ExitStack

import numpy as np

import bass_rust
import concourse.bass as bass
import concourse.tile as tile
from concourse import mybir
from concourse.bass_utils import run_bass_kernel_spmd

N_CORES = 8
B = 64          # samples per core
C, T, V, E = 3, 128, 25, 64
VV = V * V
HR = 13         # rows per half (h=1 row 12 is a zero pad)
HF = HR * V     # free elems per half (325)
ND = 25         # death slots per sample (24 real + sentinel)
DT = mybir.dt.float32
F16 = mybir.dt.float16
GMIN = 1e-6     # sqrt(1e-12): the reference's global min (diagonal), exact
BIG = 60000.0   # fp16-representable sentinel; exp(-U(sqrt(BIG)-C)^2) == 0
VSP = 19        # DVE/GpSimd free-dim split (rate ratio ~1.25 : 4.0 ns/elem)


def _split_excess_waits(nc, cap=1):
    """The walrus build in this env rejects instructions carrying more than
    ~2 semaphore-wait commands. Move excess waits onto same-engine NOPs
    inserted immediately before the offending instruction."""
    n_split = 0
    for bb in nc.main_func.blocks:
        insts = bb.instructions
        i = 0
        while i < len(insts):
            ins = insts[i]
            si = ins.sync_info
            waits = list(si.on_wait) if si and si.on_wait else []
            if len(waits) > cap:
                extra, keep = waits[:-cap], waits[-cap:]
                ins.sync_info = mybir.SyncInfo(
                    on_wait=keep, on_update=list(si.on_update or [])
                )
                for j, w in enumerate(extra):
                    nop = bass_rust.InstNoOp(
                        name=f"I-wsplit-{n_split}-{j}",
                        engine=ins.engine,
                        sync_info=mybir.SyncInfo(on_wait=[w], on_update=[]),
                    )
                    insts.insert(i, nop)
                    i += 1
                n_split += 1
            i += 1
    return n_split


def _build_pass1():
    A = mybir.AluOpType
    ACT = mybir.ActivationFunctionType
    nc = bass.Bass("TRN2", debug=False, num_devices=N_CORES)

    x_in = nc.dram_tensor("x", [B, C, T, V], F16, kind="ExternalInput").ap()
    pm_in = nc.dram_tensor("pm", [128, B], F16, kind="ExternalInput").ap()
    ut_in = nc.dram_tensor("ut2", [128, HF], F16, kind="ExternalInput").ap()
    dup_in = nc.dram_tensor("dup", [128, 128], F16, kind="ExternalInput").ap()
    id_in = nc.dram_tensor("id64", [128, B], F16, kind="ExternalInput").ap()
    dth_d = nc.dram_tensor("deaths", [B, ND], DT, kind="ExternalOutput").ap()
    pmx_d = nc.dram_tensor("pmax", [128, 1], F16, kind="ExternalOutput").ap()

    with tile.TileContext(nc, num_cores=N_CORES) as tc, ExitStack() as ctx:
        sb = ctx.enter_context(tc.tile_pool(name="sb", bufs=1))
        psum = ctx.enter_context(tc.tile_pool(name="psum", bufs=1, space="PSUM"))
        psr = ctx.enter_context(tc.tile_pool(name="psr", bufs=2, space="PSUM"))

        # ---- x DMA on 4 queues: xa first (tree starts on it), then xb ----
        xa = sb.tile([128, C, T // 4, V], F16)
        xb = sb.tile([128, C, T // 4, V], F16)
        nc.sync.dma_start(xa[0:B], x_in[:, :, 0:32, :])
        nc.scalar.dma_start(xa[B:128], x_in[:, :, 64:96, :])
        nc.sync.dma_start(xb[0:B], x_in[:, :, 32:64, :])
        nc.scalar.dma_start(xb[B:128], x_in[:, :, 96:128, :])

        # ---- small constant loads (after x) ----
        pm_t = sb.tile([128, B], F16)
        nc.sync.dma_start(pm_t[:], pm_in[:])
        ut2 = sb.tile([128, HF], F16)
        nc.scalar.dma_start(ut2[:], ut_in[:])
        dup16 = sb.tile([128, 128], F16)
        nc.sync.dma_start(dup16[:], dup_in[:])
        id16 = sb.tile([128, B], F16)
        nc.scalar.dma_start(id16[:], id_in[:])

        # ---- preload the sqrt activation table during the DMA gap ----
        warm = sb.tile([1, 2], DT)
        nc.vector.memset(warm[:], 1.0)
        nc.scalar.activation(warm[:], warm[:], ACT.Sqrt, bias=0.0, scale=1.0)

        # ---- mean over T: all-DVE fp16 add tree (2x packed mode) ----
        for xh in (xa, xb):
            for w in (16, 8, 4, 2, 1):
                nc.vector.tensor_tensor(
                    out=xh[:, :, 0:w, :],
                    in0=xh[:, :, 0:w, :],
                    in1=xh[:, :, w : 2 * w, :],
                    op=A.add,
                )
        nc.vector.tensor_tensor(
            out=xa[:, :, 0:1, :], in0=xa[:, :, 0:1, :], in1=xb[:, :, 0:1, :],
            op=A.add,
        )
        ps_xm = psum.tile([B, C, V], DT)
        nc.tensor.matmul(out=ps_xm[:], lhsT=pm_t[:], rhs=xa[:, :, 0, :],
                         start=True, stop=True)
        xm = sb.tile([B, C, V], F16)
        nc.vector.tensor_copy(xm[:], ps_xm[:])

        # ---- distance^2 matrix, fp16 (no sqrt needed before the deaths) ----
        df = sb.tile([B, C, V, V], F16)
        xmb_i = xm.unsqueeze(-1).broadcast_to([B, C, V, V])
        xmb_j = xm.unsqueeze(2).broadcast_to([B, C, V, V])
        nc.vector.tensor_tensor(out=df[:], in0=xmb_i[:], in1=xmb_j[:], op=A.subtract)
        nc.vector.tensor_tensor(out=df[:], in0=df[:], in1=df[:], op=A.mult)
        d12 = sb.tile([B, VV], F16)
        d123 = d12.rearrange("p (i j) -> p i j", i=V)
        nc.vector.tensor_tensor(out=d123[:], in0=df[:, 0], in1=df[:, 1], op=A.add)
        dq64 = sb.tile([B, 2 * HF], F16)
        nc.vector.memset(dq64[:, VV : 2 * HF], 0.0)
        dq643 = dq64[:, 0:VV].rearrange("p (i j) -> p i j", i=V)
        nc.vector.tensor_tensor(out=dq643[:], in0=d123[:], in1=df[:, 2], op=A.add)

        # ---- split to [128, 325]: partitions (h*64+b), rows h*13..h*13+12 ----
        ps_dq = psum.tile([128, HF], DT)
        nc.tensor.matmul(out=ps_dq[0:B, :], lhsT=id16[0:B], rhs=dq64[:, 0:HF],
                         start=True, stop=True)
        nc.tensor.matmul(out=ps_dq[B:128, :], lhsT=id16[0:B], rhs=dq64[:, HF : 2 * HF],
                         start=True, stop=True)
        dq = sb.tile([128, HF], F16)
        nc.vector.tensor_copy(dq[:], ps_dq[:])
        M = sb.tile([128, HF], F16)
        nc.vector.tensor_copy(M[:], dq[:])

        # ---- Floyd-Warshall min-max closure, fp16, PE row-pivot extract.
        # Per step: one 1x broadcast max, one tiny strip-copy min (feeds the
        # next pivot's PE broadcast without a WAR on M), one full 2x min.
        M3 = M.rearrange("p (i j) -> p i j", i=HR)
        fwt = sb.tile([128, HR, V], F16)
        stripc = sb.tile([128, V], F16)
        rowk0 = psr.tile([128, V], DT, tag="rowk0")
        rowk1 = psr.tile([128, V], DT, tag="rowk1")
        rowk = [rowk0, rowk1]
        for k in range(V):
            hk, ilk = divmod(k, HR)
            rk = rowk[k % 2]
            if k == 0:
                rhs = M3[0:B, 0, :]
            else:
                rhs = stripc[hk * B : (hk + 1) * B, :]
            nc.tensor.matmul(out=rk[:], lhsT=dup16[hk * B : (hk + 1) * B, :],
                             rhs=rhs, start=True, stop=True)
            nc.vector.tensor_tensor(
                out=fwt[:],
                in0=M3[:, :, k : k + 1].broadcast_to([128, HR, V]),
                in1=rk.unsqueeze(1).broadcast_to([128, HR, V]),
                op=A.max,
            )
            if k < V - 1:
                hn, iln = divmod(k + 1, HR)
                nc.vector.tensor_tensor(
                    out=stripc[hn * B : (hn + 1) * B, :],
                    in0=M3[hn * B : (hn + 1) * B, iln, :],
                    in1=fwt[hn * B : (hn + 1) * B, iln, :],
                    op=A.min,
                )
            nc.vector.tensor_tensor(out=M3[:], in0=M3[:], in1=fwt[:], op=A.min)

        # ---- premask, MST mask, masked values ----
        dut = sb.tile([128, HF], F16)
        nc.vector.tensor_tensor(out=dut[:], in0=dq[:], in1=ut2[:], op=A.mult)
        mk = sb.tile([128, HF], F16)
        nc.vector.tensor_tensor(out=mk[:], in0=M[:], in1=dq[:], op=A.is_ge)
        val = sb.tile([128, HF], F16)
        nc.vector.tensor_tensor(out=val[:], in0=mk[:], in1=dut[:], op=A.mult)

        # ---- per-half top-24, merge, top-25 ----
        d24h = sb.tile([128, 24], F16)
        mr1 = sb.tile([128, HF], F16)
        mr2 = sb.tile([128, HF], F16)
        nc.vector.max(d24h[:, 0:8], val[:])
        nc.vector.match_replace(mr1[:], d24h[:, 0:8], val[:], 0.0)
        nc.vector.max(d24h[:, 8:16], mr1[:])
        nc.vector.match_replace(mr2[:], d24h[:, 8:16], mr1[:], 0.0)
        nc.vector.max(d24h[:, 16:24], mr2[:])

        ps_c = psum.tile([B, 48], DT)
        nc.tensor.matmul(out=ps_c[:, 0:24], lhsT=id16[0:B], rhs=d24h[0:B, :],
                         start=True, stop=True)
        nc.tensor.matmul(out=ps_c[:, 24:48], lhsT=id16[B:128], rhs=d24h[B:128, :],
                         start=True, stop=True)
        cand = sb.tile([B, 48], F16)
        nc.vector.tensor_copy(cand[:], ps_c[:])

        d25 = sb.tile([B, 32], F16)
        cr1 = sb.tile([B, 48], F16)
        cr2 = sb.tile([B, 48], F16)
        cr3 = sb.tile([B, 48], F16)
        nc.vector.max(d25[:, 0:8], cand[:])
        nc.vector.match_replace(cr1[:], d25[:, 0:8], cand[:], 0.0)
        nc.vector.max(d25[:, 8:16], cr1[:])
        nc.vector.match_replace(cr2[:], d25[:, 8:16], cr1[:], 0.0)
        nc.vector.max(d25[:, 16:24], cr2[:])
        nc.vector.match_replace(cr3[:], d25[:, 16:24], cr2[:], 0.0)
        nc.vector.max(d25[:, 24:32], cr3[:])

        # ---- fp16-tie dedup: when a 25th nonzero exists, kill adjacent
        # duplicates with a +BIG sentinel; then map empty slots to BIG ----
        bigt = sb.tile([B, 1], DT)
        nc.vector.memset(bigt[:], BIG)
        zt = sb.tile([B, 1], F16)
        nc.vector.memset(zt[:], 0.0)
        g32 = sb.tile([B, 1], DT)
        nc.vector.tensor_tensor(out=g32[:], in0=d25[:, 24:25], in1=zt[:], op=A.is_gt)
        eq = sb.tile([B, 24], DT)
        nc.vector.tensor_tensor(out=eq[:], in0=d25[:, 0:24], in1=d25[:, 1:25],
                                op=A.is_equal)
        nc.vector.tensor_scalar(out=eq[:], in0=eq[:], scalar1=g32[:, 0:1],
                                scalar2=None, op0=A.mult)
        nc.vector.scalar_tensor_tensor(
            out=d25[:, 0:24], in0=eq[:], scalar=bigt[:, 0:1], in1=d25[:, 0:24],
            op0=A.mult, op1=A.add,
        )
        zb = sb.tile([B, ND], DT)
        nc.vector.tensor_scalar(out=zb[:], in0=d25[:, 0:ND], scalar1=0.0,
                                scalar2=None, op0=A.is_equal)
        nc.vector.scalar_tensor_tensor(
            out=d25[:, 0:ND], in0=zb[:], scalar=bigt[:, 0:1], in1=d25[:, 0:ND],
            op0=A.mult, op1=A.add,
        )

        # ---- deaths = sqrt(selected d^2), fp32 out ----
        dth = sb.tile([B, ND], DT)
        nc.scalar.activation(dth[:], d25[:, 0:ND], ACT.Sqrt, bias=0.0, scale=1.0)
        nc.sync.dma_start(dth_d[:], dth[:])

        # ---- per-half max of d^2 (host folds into the global max) ----
        pmx = sb.tile([128, 1], F16)
        nc.vector.tensor_reduce(out=pmx[:], in_=dq[:],
                                axis=mybir.AxisListType.X, op=A.max)
        nc.scalar.dma_start(pmx_d[:], pmx[:])

    _split_excess_waits(nc)
    return nc


def _build_pass2():
    A = mybir.AluOpType
    ACT = mybir.ActivationFunctionType
    nc = bass.Bass("TRN2", debug=False, num_devices=N_CORES)

    dth_in = nc.dram_tensor("deaths", [B, ND], DT, kind="ExternalInput").ap()
    prm_in = nc.dram_tensor("prm2", [128, 2 * (E // 2)], DT, kind="ExternalInput").ap()
    ab_in = nc.dram_tensor("ab2", [128, E // 2], DT, kind="ExternalInput").ap()
    out_d = nc.dram_tensor("out", [128, E // 2], DT, kind="ExternalOutput").ap()

    EH = E // 2  # structure elements per partition half (32)

    with tile.TileContext(nc, num_cores=N_CORES) as tc, ExitStack() as ctx:
        sb = ctx.enter_context(tc.tile_pool(name="sb", bufs=1))
        work = ctx.enter_context(tc.tile_pool(name="work", bufs=2))
        psum = ctx.enter_context(tc.tile_pool(name="psum", bufs=1, space="PSUM"))

        # deaths duplicated into both partition halves straight from DRAM
        dthb = sb.tile([128, ND], DT)
        nc.sync.dma_start(dthb[0:B], dth_in[:])
        nc.scalar.dma_start(dthb[B:128], dth_in[:])
        # params pre-broadcast on host: [128, (C_e | U_e)] per half
        prm = sb.tile([128, 2 * EH], DT)
        nc.sync.dma_start(prm[:], prm_in[:])
        ab2 = sb.tile([128, EH], DT)
        nc.scalar.dma_start(ab2[:], ab_in[:])
        warm = sb.tile([1, 2], DT)
        nc.vector.memset(warm[:], 1.0)
        nc.scalar.activation(warm[:], warm[:], ACT.Exp, bias=0.0, scale=-1.0)

        # structure element layer on [128, EH/2, ND] sub-chunks (e-halves on
        # partition halves): s = U_e (death_p - C_e)^2
        S = sb.tile([128, EH], DT)
        ECH = EH // 2
        for ch in range(2):
            e0 = ch * ECH
            t1 = work.tile([128, ECH, ND], DT, tag="t1")
            nc.vector.tensor_tensor(
                out=t1[:],
                in0=dthb.unsqueeze(1).broadcast_to([128, ECH, ND]),
                in1=prm[:, e0 : e0 + ECH].unsqueeze(-1).broadcast_to([128, ECH, ND]),
                op=A.subtract,
            )
            nc.scalar.square(t1[:], t1[:])
            nc.vector.tensor_tensor(
                out=t1[:],
                in0=t1[:],
                in1=prm[:, EH + e0 : EH + e0 + ECH].unsqueeze(-1)
                .broadcast_to([128, ECH, ND]),
                op=A.mult,
            )
            fexp = work.tile([128, ECH, ND], DT, tag="fexp")
            nc.scalar.activation(fexp[:], t1[:], ACT.Exp, bias=0.0, scale=-1.0)
            nc.vector.tensor_reduce(
                out=S[:, e0 : e0 + ECH], in_=fexp[:], axis=mybir.AxisListType.X,
                op=A.add,
            )
        outt = sb.tile([128, EH], DT)
        nc.vector.tensor_tensor(out=outt[:], in0=S[:], in1=ab2[:], op=A.mult)
        nc.sync.dma_start(out_d[:], outt[:])

    _split_excess_waits(nc)
    return nc


_CACHE = {}


def _consts():
    # pair matrix: adds partition rows b and b+64 (the two T-halves) and
    # applies the 1/T mean scale
    pairmat = np.zeros((128, B), dtype=np.float16)
    for p in range(128):
        pairmat[p, p % B] = 1.0 / T
    # upper-tri premask in the split layout: partition (h*64+b) holds rows
    # i = h*13 .. h*13+12; pad row (h=1, il=12) is zero
    ut2 = np.zeros((128, HF), dtype=np.float16)
    for h in range(2):
        for il in range(HR):
            i = h * HR + il
            if i >= V:
                continue
            for j in range(V):
                if j > i:
                    ut2[h * B : (h + 1) * B, il * V + j] = 1.0
    dup = np.zeros((128, 128), dtype=np.float16)
    for p in range(128):
        dup[p % B, p] = 1.0
        dup[B + p % B, p] = 1.0
    id64 = np.concatenate([np.eye(B, dtype=np.float16)] * 2, axis=0)
    return pairmat, ut2, dup, id64


def _get_programs():
    if "p1" not in _CACHE:
        _CACHE["p1"] = _build_pass1()
        _CACHE["p2"] = _build_pass2()
    return _CACHE["p1"], _CACHE["p2"]


def _run(x, centres, sharpness, **run_kwargs):
    p1, p2 = _get_programs()
    xf = np.ascontiguousarray(x.reshape(-1, C, T, V)).astype(np.float16)
    n_total = xf.shape[0]
    assert n_total == N_CORES * B, xf.shape
    pairmat, ut2, dup, id64 = _consts()

    in1 = [
        {
            "x": np.ascontiguousarray(xf[i * B : (i + 1) * B]),
            "pm": pairmat,
            "ut2": ut2,
            "dup": dup,
            "id64": id64,
        }
        for i in range(N_CORES)
    ]
    res1 = run_bass_kernel_spmd(p1, in1, list(range(N_CORES)), **run_kwargs)

    # host: fold the global max into the structure-element parameters
    gmax2 = max(
        float(np.max(res1.results[i]["pmax"].astype(np.float32)))
        for i in range(N_CORES)
    )
    gmax = float(np.sqrt(gmax2))
    R = gmax - GMIN
    c1 = centres[:, 0].astype(np.float64)
    c2 = centres[:, 1].astype(np.float64)
    s1 = sharpness[:, 0].astype(np.float64)
    s2 = sharpness[:, 1].astype(np.float64)
    Ce = (GMIN + c2 * R).astype(np.float32)
    Ue = ((s2 / R) ** 2).astype(np.float32)
    Abe = np.exp(-((s1 * c1) ** 2)).astype(np.float32)
    EH = E // 2
    prm2 = np.zeros((128, 2 * EH), dtype=np.float32)
    prm2[0:B, 0:EH] = Ce[0:EH]
    prm2[B:128, 0:EH] = Ce[EH:E]
    prm2[0:B, EH : 2 * EH] = Ue[0:EH]
    prm2[B:128, EH : 2 * EH] = Ue[EH:E]
    ab2 = np.zeros((128, EH), dtype=np.float32)
    ab2[0:B] = Abe[0:EH]
    ab2[B:128] = Abe[EH:E]

    in2 = [
        {
            "deaths": np.ascontiguousarray(res1.results[i]["deaths"]),
            "prm2": prm2,
            "ab2": ab2,
        }
        for i in range(N_CORES)
    ]
    res2 = run_bass_kernel_spmd(p2, in2, list(range(N_CORES)), **run_kwargs)

    # unshard: out[b, e] with e<32 on partition b, e>=32 on partition 64+b
    out = np.concatenate(
        [
            np.concatenate(
                [res2.results[i]["out"][0:B], res2.results[i]["out"][B:128]], axis=1
            )
            for i in range(N_CORES)
        ],
        axis=0,
    )
    return out, (res1, res2)


def kernel(x, centres, sharpness):
    out, _ = _run(np.asarray(x), np.asarray(centres), np.asarray(sharpness))
    return out


# revision 11
# speedup vs baseline: 1.2252x; 1.0144x over previous
"""TopoEncoder Trainium2 kernel (8 NeuronCores, data-parallel over batch).

Two-pass design — the reference's single global scalar (max over the whole
batch's distance tensor) is the only cross-core quantity, and the CC-stream
collective path costs ~65us of pure latency on these axon-tunneled cores
(43.6us kernel-entry barrier + 11us stream gap + 9.5us mesh op). Instead:

  pass 1 (per core, 64 samples): x DMA -> mean over T (DVE/GpSimd add-tree +
    PE pair-matrix fold) -> pairwise channel-L2 distance^2, cast fp16 ->
    split each sample's 25x25 across two partitions ([128, 13*25], PE
    identity matmuls) -> Floyd-Warshall min-max closure in fp16 with PE
    row-pivot extraction (selection ops only, so fp16 rounds each d^2 once)
    -> MST mask (M >= d, exact on fp16 values) -> per-half top-24 of the
    masked upper-tri (max8 + match_replace) -> merge halves (PE) -> top-25
    -> kill the first adjacent duplicate and zero slots with a +BIG sentinel
    (fp16 ties can admit a 25th spurious mask edge whose value duplicates a
    real one; measured output rel err ~1e-5 after this) -> sqrt ->
    deaths [64,25] f32, plus the per-half max of d^2 [128,1].
  host: folds gmax = sqrt(max of the 1024 per-half maxima) into the
    structure-element parameters (pure parameter prep, like the baseline's
    csT/pairmat): C_e = 1e-6 + c2_e*R, U_e = (s2_e/R)^2, Ab_e = exp(-(s1 c1)^2)
    with R = gmax - 1e-6 (global min is the d=sqrt(1e-12) diagonal,
    structurally 1e-6).
  pass 2 (per core): normalize-free structure-element layer on deaths with
    the folded params: out[b,e] = Ab_e * sum_p exp(-U_e (death_p - C_e)^2).
    BIG slots underflow exp to exactly 0.

fp16 matters because DVE's 2x mode needs 2-byte dtypes with packed innermost
access; the FW min and the mask all hit it (the col-broadcast max stays at
1x — access-pattern-bound, not ALU-bound — which is why the [128, 13*25]
split layout halves its free size).
"""

from contextlib import ExitStack

import numpy as np

import bass_rust
import concourse.bass as bass
import concourse.tile as tile
from concourse import mybir
from concourse.bass_utils import run_bass_kernel_spmd

N_CORES = 8
B = 64          # samples per core
C, T, V, E = 3, 128, 25, 64
VV = V * V
HR = 13         # rows per half (h=1 row 12 is a zero pad)
HF = HR * V     # free elems per half (325)
ND = 25         # death slots per sample (24 real + sentinel)
DT = mybir.dt.float32
F16 = mybir.dt.float16
GMIN = 1e-6     # sqrt(1e-12): the reference's global min (diagonal), exact
BIG = 60000.0   # fp16-representable sentinel; exp(-U(sqrt(BIG)-C)^2) == 0
VSP = 19        # DVE/GpSimd free-dim split (rate ratio ~1.25 : 4.0 ns/elem)


def _split_excess_waits(nc, cap=1):
    """The walrus build in this env rejects instructions carrying more than
    ~2 semaphore-wait commands. Move excess waits onto same-engine NOPs
    inserted immediately before the offending instruction."""
    n_split = 0
    for bb in nc.main_func.blocks:
        insts = bb.instructions
        i = 0
        while i < len(insts):
            ins = insts[i]
            si = ins.sync_info
            waits = list(si.on_wait) if si and si.on_wait else []
            if len(waits) > cap:
                extra, keep = waits[:-cap], waits[-cap:]
                ins.sync_info = mybir.SyncInfo(
                    on_wait=keep, on_update=list(si.on_update or [])
                )
                for j, w in enumerate(extra):
                    nop = bass_rust.InstNoOp(
                        name=f"I-wsplit-{n_split}-{j}",
                        engine=ins.engine,
                        sync_info=mybir.SyncInfo(on_wait=[w], on_update=[]),
                    )
                    insts.insert(i, nop)
                    i += 1
                n_split += 1
            i += 1
    return n_split


def _build_pass1():
    A = mybir.AluOpType
    ACT = mybir.ActivationFunctionType
    nc = bass.Bass("TRN2", debug=False, num_devices=N_CORES)

    x_in = nc.dram_tensor("x", [B, C, T, V], F16, kind="ExternalInput").ap()
    pm_in = nc.dram_tensor("pm", [128, B], F16, kind="ExternalInput").ap()
    ut_in = nc.dram_tensor("ut2", [128, HF], F16, kind="ExternalInput").ap()
    dup_in = nc.dram_tensor("dup", [128, 128], F16, kind="ExternalInput").ap()
    id_in = nc.dram_tensor("id64", [128, B], F16, kind="ExternalInput").ap()
    dth_d = nc.dram_tensor("deaths", [B, ND], DT, kind="ExternalOutput").ap()
    pmx_d = nc.dram_tensor("pmax", [128, 1], F16, kind="ExternalOutput").ap()

    with tile.TileContext(nc, num_cores=N_CORES) as tc, ExitStack() as ctx:
        sb = ctx.enter_context(tc.tile_pool(name="sb", bufs=1))
        psum = ctx.enter_context(tc.tile_pool(name="psum", bufs=1, space="PSUM"))
        psr = ctx.enter_context(tc.tile_pool(name="psr", bufs=2, space="PSUM"))

        # ---- x DMA on 4 queues: xa first (tree starts on it), then xb ----
        xa = sb.tile([128, C, T // 4, V], F16)
        xb = sb.tile([128, C, T // 4, V], F16)
        nc.sync.dma_start(xa[0:B], x_in[:, :, 0:32, :])
        nc.scalar.dma_start(xa[B:128], x_in[:, :, 64:96, :])
        nc.sync.dma_start(xb[0:B], x_in[:, :, 32:64, :])
        nc.scalar.dma_start(xb[B:128], x_in[:, :, 96:128, :])

        # ---- small constant loads (after x) ----
        pm_t = sb.tile([128, B], F16)
        nc.sync.dma_start(pm_t[:], pm_in[:])
        ut2 = sb.tile([128, HF], F16)
        nc.scalar.dma_start(ut2[:], ut_in[:])
        dup16 = sb.tile([128, 128], F16)
        nc.sync.dma_start(dup16[:], dup_in[:])
        id16 = sb.tile([128, B], F16)
        nc.scalar.dma_start(id16[:], id_in[:])

        # ---- preload the sqrt activation table during the DMA gap ----
        warm = sb.tile([1, 2], DT)
        nc.vector.memset(warm[:], 1.0)
        nc.scalar.activation(warm[:], warm[:], ACT.Sqrt, bias=0.0, scale=1.0)

        # ---- mean over T: all-DVE fp16 add tree (2x packed mode) ----
        for xh in (xa, xb):
            for w in (16, 8, 4, 2, 1):
                nc.vector.tensor_tensor(
                    out=xh[:, :, 0:w, :],
                    in0=xh[:, :, 0:w, :],
                    in1=xh[:, :, w : 2 * w, :],
                    op=A.add,
                )
        nc.vector.tensor_tensor(
            out=xa[:, :, 0:1, :], in0=xa[:, :, 0:1, :], in1=xb[:, :, 0:1, :],
            op=A.add,
        )
        ps_xm = psum.tile([B, C, V], DT)
        nc.tensor.matmul(out=ps_xm[:], lhsT=pm_t[:], rhs=xa[:, :, 0, :],
                         start=True, stop=True)
        xm = sb.tile([B, C, V], F16)
        nc.vector.tensor_copy(xm[:], ps_xm[:])

        # ---- distance^2 matrix, fp16 (no sqrt needed before the deaths) ----
        df = sb.tile([B, C, V, V], F16)
        xmb_i = xm.unsqueeze(-1).broadcast_to([B, C, V, V])
        xmb_j = xm.unsqueeze(2).broadcast_to([B, C, V, V])
        nc.vector.tensor_tensor(out=df[:], in0=xmb_i[:], in1=xmb_j[:], op=A.subtract)
        sq = sb.tile([B, C, V, V], F16)
        nc.vector.tensor_tensor(out=sq[:], in0=df[:], in1=df[:], op=A.mult)
        d12 = sb.tile([B, VV], F16)
        d123 = d12.rearrange("p (i j) -> p i j", i=V)
        nc.vector.tensor_tensor(out=d123[:], in0=sq[:, 0], in1=sq[:, 1], op=A.add)
        dq64 = sb.tile([B, 2 * HF], F16)
        nc.vector.memset(dq64[:, VV : 2 * HF], 0.0)
        dq643 = dq64[:, 0:VV].rearrange("p (i j) -> p i j", i=V)
        nc.vector.tensor_tensor(out=dq643[:], in0=d123[:], in1=sq[:, 2], op=A.add)

        # ---- split to [128, 325]: partitions (h*64+b), rows h*13..h*13+12 ----
        ps_dq = psum.tile([128, HF], DT)
        nc.tensor.matmul(out=ps_dq[0:B, :], lhsT=id16[0:B], rhs=dq64[:, 0:HF],
                         start=True, stop=True)
        nc.tensor.matmul(out=ps_dq[B:128, :], lhsT=id16[0:B], rhs=dq64[:, HF : 2 * HF],
                         start=True, stop=True)
        dq = sb.tile([128, HF], F16)
        nc.vector.tensor_copy(dq[:], ps_dq[:])
        M = sb.tile([128, HF], F16)
        nc.vector.tensor_copy(M[:], dq[:])

        # ---- Floyd-Warshall min-max closure, fp16, PE row-pivot extract.
        # Per step: one 1x broadcast max, one tiny strip-copy min (feeds the
        # next pivot's PE broadcast without a WAR on M), one full 2x min.
        M3 = M.rearrange("p (i j) -> p i j", i=HR)
        fwt = sb.tile([128, HR, V], F16)
        stripc = sb.tile([128, V], F16)
        rowk0 = psr.tile([128, V], DT, tag="rowk0")
        rowk1 = psr.tile([128, V], DT, tag="rowk1")
        rowk = [rowk0, rowk1]
        for k in range(V):
            hk, ilk = divmod(k, HR)
            rk = rowk[k % 2]
            if k == 0:
                rhs = M3[0:B, 0, :]
            else:
                rhs = stripc[hk * B : (hk + 1) * B, :]
            nc.tensor.matmul(out=rk[:], lhsT=dup16[hk * B : (hk + 1) * B, :],
                             rhs=rhs, start=True, stop=True)
            nc.vector.tensor_tensor(
                out=fwt[:],
                in0=M3[:, :, k : k + 1].broadcast_to([128, HR, V]),
                in1=rk.unsqueeze(1).broadcast_to([128, HR, V]),
                op=A.max,
            )
            if k < V - 1:
                hn, iln = divmod(k + 1, HR)
                nc.vector.tensor_tensor(
                    out=stripc[hn * B : (hn + 1) * B, :],
                    in0=M3[hn * B : (hn + 1) * B, iln, :],
                    in1=fwt[hn * B : (hn + 1) * B, iln, :],
                    op=A.min,
                )
            nc.vector.tensor_tensor(out=M3[:], in0=M3[:], in1=fwt[:], op=A.min)

        # ---- premask, MST mask, masked values ----
        dut = sb.tile([128, HF], F16)
        nc.vector.tensor_tensor(out=dut[:], in0=dq[:], in1=ut2[:], op=A.mult)
        mk = sb.tile([128, HF], F16)
        nc.vector.tensor_tensor(out=mk[:], in0=M[:], in1=dq[:], op=A.is_ge)
        val = sb.tile([128, HF], F16)
        nc.vector.tensor_tensor(out=val[:], in0=mk[:], in1=dut[:], op=A.mult)

        # ---- per-half top-24, merge, top-25 ----
        d24h = sb.tile([128, 24], F16)
        mr1 = sb.tile([128, HF], F16)
        mr2 = sb.tile([128, HF], F16)
        nc.vector.max(d24h[:, 0:8], val[:])
        nc.vector.match_replace(mr1[:], d24h[:, 0:8], val[:], 0.0)
        nc.vector.max(d24h[:, 8:16], mr1[:])
        nc.vector.match_replace(mr2[:], d24h[:, 8:16], mr1[:], 0.0)
        nc.vector.max(d24h[:, 16:24], mr2[:])

        ps_c = psum.tile([B, 48], DT)
        nc.tensor.matmul(out=ps_c[:, 0:24], lhsT=id16[0:B], rhs=d24h[0:B, :],
                         start=True, stop=True)
        nc.tensor.matmul(out=ps_c[:, 24:48], lhsT=id16[B:128], rhs=d24h[B:128, :],
                         start=True, stop=True)
        cand = sb.tile([B, 48], F16)
        nc.vector.tensor_copy(cand[:], ps_c[:])

        d25 = sb.tile([B, 32], F16)
        cr1 = sb.tile([B, 48], F16)
        cr2 = sb.tile([B, 48], F16)
        cr3 = sb.tile([B, 48], F16)
        nc.vector.max(d25[:, 0:8], cand[:])
        nc.vector.match_replace(cr1[:], d25[:, 0:8], cand[:], 0.0)
        nc.vector.max(d25[:, 8:16], cr1[:])
        nc.vector.match_replace(cr2[:], d25[:, 8:16], cr1[:], 0.0)
        nc.vector.max(d25[:, 16:24], cr2[:])
        nc.vector.match_replace(cr3[:], d25[:, 16:24], cr2[:], 0.0)
        nc.vector.max(d25[:, 24:32], cr3[:])

        # ---- fp16-tie dedup: when a 25th nonzero exists, kill adjacent
        # duplicates with a +BIG sentinel; then map empty slots to BIG ----
        bigt = sb.tile([B, 1], DT)
        nc.vector.memset(bigt[:], BIG)
        zt = sb.tile([B, 1], F16)
        nc.vector.memset(zt[:], 0.0)
        g32 = sb.tile([B, 1], DT)
        nc.vector.tensor_tensor(out=g32[:], in0=d25[:, 24:25], in1=zt[:], op=A.is_gt)
        eq = sb.tile([B, 24], DT)
        nc.vector.tensor_tensor(out=eq[:], in0=d25[:, 0:24], in1=d25[:, 1:25],
                                op=A.is_equal)
        nc.vector.tensor_scalar(out=eq[:], in0=eq[:], scalar1=g32[:, 0:1],
                                scalar2=None, op0=A.mult)
        nc.vector.scalar_tensor_tensor(
            out=d25[:, 0:24], in0=eq[:], scalar=bigt[:, 0:1], in1=d25[:, 0:24],
            op0=A.mult, op1=A.add,
        )
        zb = sb.tile([B, ND], DT)
        nc.vector.tensor_scalar(out=zb[:], in0=d25[:, 0:ND], scalar1=0.0,
                                scalar2=None, op0=A.is_equal)
        nc.vector.scalar_tensor_tensor(
            out=d25[:, 0:ND], in0=zb[:], scalar=bigt[:, 0:1], in1=d25[:, 0:ND],
            op0=A.mult, op1=A.add,
        )

        # ---- deaths = sqrt(selected d^2), fp32 out ----
        dth = sb.tile([B, ND], DT)
        nc.scalar.activation(dth[:], d25[:, 0:ND], ACT.Sqrt, bias=0.0, scale=1.0)
        nc.sync.dma_start(dth_d[:], dth[:])

        # ---- per-half max of d^2 (host folds into the global max) ----
        pmx = sb.tile([128, 1], F16)
        nc.vector.tensor_reduce(out=pmx[:], in_=dq[:],
                                axis=mybir.AxisListType.X, op=A.max)
        nc.scalar.dma_start(pmx_d[:], pmx[:])

    _split_excess_waits(nc)
    return nc


def _build_pass2():
    A = mybir.AluOpType
    ACT = mybir.ActivationFunctionType
    nc = bass.Bass("TRN2", debug=False, num_devices=N_CORES)

    dth_in = nc.dram_tensor("deaths", [B, ND], DT, kind="ExternalInput").ap()
    prm_in = nc.dram_tensor("prm2", [128, 2 * (E // 2)], DT, kind="ExternalInput").ap()
    ab_in = nc.dram_tensor("ab2", [128, E // 2], DT, kind="ExternalInput").ap()
    out_d = nc.dram_tensor("out", [128, E // 2], DT, kind="ExternalOutput").ap()

    EH = E // 2  # structure elements per partition half (32)

    with tile.TileContext(nc, num_cores=N_CORES) as tc, ExitStack() as ctx:
        sb = ctx.enter_context(tc.tile_pool(name="sb", bufs=1))
        work = ctx.enter_context(tc.tile_pool(name="work", bufs=2))
        psum = ctx.enter_context(tc.tile_pool(name="psum", bufs=1, space="PSUM"))

        # deaths duplicated into both partition halves straight from DRAM
        warm = sb.tile([1, 2], DT)
        nc.vector.memset(warm[:], 1.0)
        nc.scalar.activation(warm[:], warm[:], ACT.Exp, bias=0.0, scale=-1.0)
        dthb = sb.tile([128, ND], DT)
        nc.sync.dma_start(dthb[0:B], dth_in[:])
        nc.sync.dma_start(dthb[B:128], dth_in[:])
        # params pre-broadcast on host: [128, (C_e | U_e)] per half
        prm = sb.tile([128, 2 * EH], DT)
        nc.sync.dma_start(prm[:], prm_in[:])
        ab2 = sb.tile([128, EH], DT)
        nc.sync.dma_start(ab2[:], ab_in[:])

        # structure element layer on [128, EH/2, ND] sub-chunks (e-halves on
        # partition halves): s = U_e (death_p - C_e)^2
        S = sb.tile([128, EH], DT)
        ECH = EH // 2
        for ch in range(2):
            e0 = ch * ECH
            t1 = work.tile([128, ECH, ND], DT, tag="t1")
            nc.vector.tensor_tensor(
                out=t1[:],
                in0=dthb.unsqueeze(1).broadcast_to([128, ECH, ND]),
                in1=prm[:, e0 : e0 + ECH].unsqueeze(-1).broadcast_to([128, ECH, ND]),
                op=A.subtract,
            )
            nc.scalar.square(t1[:], t1[:])
            nc.vector.tensor_tensor(
                out=t1[:],
                in0=t1[:],
                in1=prm[:, EH + e0 : EH + e0 + ECH].unsqueeze(-1)
                .broadcast_to([128, ECH, ND]),
                op=A.mult,
            )
            fexp = work.tile([128, ECH, ND], DT, tag="fexp")
            nc.scalar.activation(fexp[:], t1[:], ACT.Exp, bias=0.0, scale=-1.0)
            nc.vector.tensor_reduce(
                out=S[:, e0 : e0 + ECH], in_=fexp[:], axis=mybir.AxisListType.X,
                op=A.add,
            )
        outt = sb.tile([128, EH], DT)
        nc.vector.tensor_tensor(out=outt[:], in0=S[:], in1=ab2[:], op=A.mult)
        nc.sync.dma_start(out_d[:], outt[:])

    _split_excess_waits(nc)
    return nc


_CACHE = {}


def _consts():
    # pair matrix: adds partition rows b and b+64 (the two T-halves) and
    # applies the 1/T mean scale
    pairmat = np.zeros((128, B), dtype=np.float16)
    for p in range(128):
        pairmat[p, p % B] = 1.0 / T
    # upper-tri premask in the split layout: partition (h*64+b) holds rows
    # i = h*13 .. h*13+12; pad row (h=1, il=12) is zero
    ut2 = np.zeros((128, HF), dtype=np.float16)
    for h in range(2):
        for il in range(HR):
            i = h * HR + il
            if i >= V:
                continue
            for j in range(V):
                if j > i:
                    ut2[h * B : (h + 1) * B, il * V + j] = 1.0
    dup = np.zeros((128, 128), dtype=np.float16)
    for p in range(128):
        dup[p % B, p] = 1.0
        dup[B + p % B, p] = 1.0
    id64 = np.concatenate([np.eye(B, dtype=np.float16)] * 2, axis=0)
    return pairmat, ut2, dup, id64


def _get_programs():
    if "p1" not in _CACHE:
        _CACHE["p1"] = _build_pass1()
        _CACHE["p2"] = _build_pass2()
    return _CACHE["p1"], _CACHE["p2"]


def _run(x, centres, sharpness, **run_kwargs):
    p1, p2 = _get_programs()
    xf = np.ascontiguousarray(x.reshape(-1, C, T, V)).astype(np.float16)
    n_total = xf.shape[0]
    assert n_total == N_CORES * B, xf.shape
    pairmat, ut2, dup, id64 = _consts()

    in1 = [
        {
            "x": np.ascontiguousarray(xf[i * B : (i + 1) * B]),
            "pm": pairmat,
            "ut2": ut2,
            "dup": dup,
            "id64": id64,
        }
        for i in range(N_CORES)
    ]
    res1 = run_bass_kernel_spmd(p1, in1, list(range(N_CORES)), **run_kwargs)

    # host: fold the global max into the structure-element parameters
    gmax2 = max(
        float(np.max(res1.results[i]["pmax"].astype(np.float32)))
        for i in range(N_CORES)
    )
    gmax = float(np.sqrt(gmax2))
    R = gmax - GMIN
    c1 = centres[:, 0].astype(np.float64)
    c2 = centres[:, 1].astype(np.float64)
    s1 = sharpness[:, 0].astype(np.float64)
    s2 = sharpness[:, 1].astype(np.float64)
    Ce = (GMIN + c2 * R).astype(np.float32)
    Ue = ((s2 / R) ** 2).astype(np.float32)
    Abe = np.exp(-((s1 * c1) ** 2)).astype(np.float32)
    EH = E // 2
    prm2 = np.zeros((128, 2 * EH), dtype=np.float32)
    prm2[0:B, 0:EH] = Ce[0:EH]
    prm2[B:128, 0:EH] = Ce[EH:E]
    prm2[0:B, EH : 2 * EH] = Ue[0:EH]
    prm2[B:128, EH : 2 * EH] = Ue[EH:E]
    ab2 = np.zeros((128, EH), dtype=np.float32)
    ab2[0:B] = Abe[0:EH]
    ab2[B:128] = Abe[EH:E]

    in2 = [
        {
            "deaths": np.ascontiguousarray(res1.results[i]["deaths"]),
            "prm2": prm2,
            "ab2": ab2,
        }
        for i in range(N_CORES)
    ]
    res2 = run_bass_kernel_spmd(p2, in2, list(range(N_CORES)), **run_kwargs)

    # unshard: out[b, e] with e<32 on partition b, e>=32 on partition 64+b
    out = np.concatenate(
        [
            np.concatenate(
                [res2.results[i]["out"][0:B], res2.results[i]["out"][B:128]], axis=1
            )
            for i in range(N_CORES)
        ],
        axis=0,
    )
    return out, (res1, res2)


def kernel(x, centres, sharpness):
    out, _ = _run(np.asarray(x), np.asarray(centres), np.asarray(sharpness))
    return out
